# revision 4
# baseline (speedup 1.0000x reference)
"""Trainium2 Bass kernel for nn_Attention_27943057228498 (sparse token-pruning
attention, ViT-style EViT) — v4.

Strategy: pure data parallelism over batch — 32 batches over 8 NeuronCores,
4 per core, no collectives.

Numerics (validated against the fixed seeded inputs on host, 0 kept-set
flips across all 32 batches, and against hw probes exp2/exp3/exp4):
  - Q/K projections: 3-term split matmuls (wh@xh + wh@xl + wl@xh) with
    fp16 hi parts and bf16 lo parts, host-prepared; exact fp32 PSUM
    accumulation gives ~2^-21-relative q/k — full precision for ranking.
  - Token-ranking diagonal: exact fp32 q*k elementwise + fp32 head-select
    matmul.
  - Scores: single fp16 matmul, exp straight to fp16.
  - Rowsums: a ones-column inside each head's 128-wide V block makes the
    attn@V matmul emit sum(p16) as PSUM row 64 in exact fp32.

PE shaping: fp16 matmuls only hit 1 cyc/col with K=128 and M in {64,128}
(measured: K=64 or M=65 run ~2x slower). So everything is padded to
K=M=128 with zeros: per-head q tiles zero the other head's 64 rows
(contraction over the full 128 channel rows then yields just head h),
V blocks are 128 wide ([v_h | 1 | 0*63]), the 65-token tail tile is
zero-padded in x/kh16/p16/o16 so its matmuls run full-width too.

Token selection without sorting: per batch, token score a_j gets rank
R_j = #{i: a_i > a_j} via compare + row-reduce; keep = R < num_kept (CLS is
forced kept by pinning a_0 = +1e30); output positions are an inclusive prefix
scan of the keep mask; rows are emitted with an indirect-DMA scatter whose
out-of-bounds indices (dropped tokens) are silently discarded.
"""

import numpy as np

import concourse.bass as bass
import concourse.bass_isa as bass_isa
import concourse.tile as tile
import concourse.mybir as mybir
from concourse import bacc
from concourse.bass_utils import run_bass_kernel_spmd

# ── problem constants ────────────────────────────────────────────────
B, N, C = 32, 577, 768
H = 12
HD = C // H              # 64
NCORES = 8
BL = B // NCORES         # 4 batches per core
SCALE = HD ** -0.5       # 0.125 (exact power of two)

P = 128
TOK_TILES = [(0, 128), (128, 128), (256, 128), (384, 128), (512, 65)]  # 577
CT = C // P              # 6 channel tiles
NPAD = 640               # 577 padded to 5*128
BIG = 1.0e9              # scatter index for dropped rows (exact in fp32)
NEG = -1.0e30            # pad value below any real score

F32 = mybir.dt.float32
F16 = mybir.dt.float16
BF16 = mybir.dt.bfloat16
U32 = mybir.dt.uint32
OP = mybir.AluOpType
ACTF = mybir.ActivationFunctionType


def _dedupe_ldweights(nc):
    """Remove back-to-back duplicate PE Ldweights (same weights AP + array
    tile) so repeated matmuls on one stationary operand pay one load."""

    def region(inst):
        tp = inst.tile_position or (0, 0)
        ts = inst.tile_size or (128, 128)
        return (tp[0], tp[0] + ts[0], tp[1], tp[1] + ts[1])

    def overlaps(r1, r2):
        return r1[0] < r2[1] and r2[0] < r1[1] and r1[2] < r2[3] and r2[2] < r1[3]

    removed = 0
    for blk in nc.m.functions[0].blocks:
        state = []   # list of (region, signature)
        keep_list = []
        for inst in blk.instructions:
            if not isinstance(inst, (mybir.InstLdweights, mybir.InstMatmult)):
                keep_list.append(inst)
                continue
            if isinstance(inst, mybir.InstLdweights):
                sig = (str(inst.ins[0]), inst.tile_position, inst.tile_size,
                       inst.is_transpose)
                r = region(inst)
                if (not inst.has_wait() and not inst.has_update()
                        and any(overlaps(r, r2) and s2 == sig and r2 == r
                                for r2, s2 in state)):
                    removed += 1
                    continue     # drop duplicate load
                state = [(r2, s2) for r2, s2 in state if not overlaps(r, r2)]
                state.append((r, sig))
                keep_list.append(inst)
            else:
                # self-loading matmul (fp32) clobbers its region
                if getattr(inst, "ldweights", None) is not False:
                    r = region(inst)
                    state = [(r2, s2) for r2, s2 in state
                             if not overlaps(r, r2)]
                keep_list.append(inst)
        if removed:
            blk.instructions[:] = keep_list
    return removed


def _chunks(total, limit=512):
    out = []
    c0 = 0
    while c0 < total:
        cn = min(limit, total - c0)
        out.append((c0, cn))
        c0 += cn
    return out


def build(num_kept: int):
    nc = bacc.Bacc("TRN2", target_bir_lowering=False, debug=False,
                   num_devices=NCORES)

    xh_e = nc.dram_tensor("xh", [BL, C, N], F16, kind="ExternalInput")
    xl_e = nc.dram_tensor("xl", [BL, C, N], BF16, kind="ExternalInput")
    xr_e = nc.dram_tensor("xr", [BL, N, C], F32, kind="ExternalInput")
    wqh_e = nc.dram_tensor("wqh", [C, C], F16, kind="ExternalInput")
    wql_e = nc.dram_tensor("wql", [C, C], BF16, kind="ExternalInput")
    wkh_e = nc.dram_tensor("wkh", [C, C], F16, kind="ExternalInput")
    wkl_e = nc.dram_tensor("wkl", [C, C], BF16, kind="ExternalInput")
    wv16_e = nc.dram_tensor("wv16", [C, C], F16, kind="ExternalInput")
    wp16_e = nc.dram_tensor("wp16", [C, C], F16, kind="ExternalInput")
    hsel_e = nc.dram_tensor("hsel", [C, H], F32, kind="ExternalInput")
    tri_e = nc.dram_tensor("tri", [P, P], F32, kind="ExternalInput")
    out_e = nc.dram_tensor("out", [BL, num_kept, C], F32, kind="ExternalOutput")
    out_flat = out_e.ap().rearrange("b n c -> (b n) c")

    from contextlib import ExitStack
    with tile.TileContext(nc) as tc, ExitStack() as ctx:
        wpool = ctx.enter_context(tc.tile_pool(name="weights", bufs=1))
        xpool = ctx.enter_context(tc.tile_pool(name="x", bufs=1))
        qkpool = ctx.enter_context(tc.tile_pool(name="qk", bufs=1))
        vpool = ctx.enter_context(tc.tile_pool(name="v", bufs=1))
        opool = ctx.enter_context(tc.tile_pool(name="o", bufs=1))
        ppool = ctx.enter_context(tc.tile_pool(name="p", bufs=2))
        spool = ctx.enter_context(tc.tile_pool(name="small", bufs=1))
        ypool = ctx.enter_context(tc.tile_pool(name="y", bufs=2))
        dpool = ctx.enter_context(tc.tile_pool(name="dram", bufs=2,
                                               space="DRAM"))
        pspool = ctx.enter_context(tc.tile_pool(name="ps", bufs=2,
                                                space="PSUM"))
        psav = ctx.enter_context(tc.tile_pool(name="psav", bufs=2,
                                              space="PSUM"))

        # ── resident weights / constants ─────────────────────────────
        wqh_t, wql_t, wkh_t, wkl_t, wv_t, wp_t, hsel_t = \
            [], [], [], [], [], [], []
        tri_box = []

        def load_weights():
            for i in range(CT):
                sl = slice(i * P, (i + 1) * P)
                w1 = wpool.tile([P, C], F16, tag=f"wkh{i}", name=f"wkh{i}")
                nc.scalar.dma_start(w1[:], wkh_e.ap()[sl, :])
                wkh_t.append(w1)
                w2 = wpool.tile([P, C], BF16, tag=f"wkl{i}", name=f"wkl{i}")
                nc.scalar.dma_start(w2[:], wkl_e.ap()[sl, :])
                wkl_t.append(w2)
            for i in range(CT):
                sl = slice(i * P, (i + 1) * P)
                w1 = wpool.tile([P, C], F16, tag=f"wqh{i}", name=f"wqh{i}")
                nc.gpsimd.dma_start(w1[:], wqh_e.ap()[sl, :])
                wqh_t.append(w1)
                w2 = wpool.tile([P, C], BF16, tag=f"wql{i}", name=f"wql{i}")
                nc.gpsimd.dma_start(w2[:], wql_e.ap()[sl, :])
                wql_t.append(w2)
            for i in range(CT):
                sl = slice(i * P, (i + 1) * P)
                w3 = wpool.tile([P, C], F16, tag=f"wv{i}", name=f"wv{i}")
                nc.gpsimd.dma_start(w3[:], wv16_e.ap()[sl, :])
                wv_t.append(w3)
            for i in range(CT):
                sl = slice(i * P, (i + 1) * P)
                w5 = wpool.tile([P, H], F32, tag=f"hs{i}", name=f"hs{i}")
                nc.gpsimd.dma_start(w5[:], hsel_e.ap()[sl, :])
                hsel_t.append(w5)
            for i in range(CT):
                sl = slice(i * P, (i + 1) * P)
                w4 = wpool.tile([P, C], F16, tag=f"wp{i}", name=f"wp{i}")
                nc.gpsimd.dma_start(w4[:], wp16_e.ap()[sl, :])
                wp_t.append(w4)
            w6 = wpool.tile([P, P], F32, tag="tri", name="tri")
            nc.gpsimd.dma_start(w6[:], tri_e.ap()[:, :])
            tri_box.append(w6)

        zrow = wpool.tile([1, NPAD], F32, tag="zrow")
        nc.vector.memset(zrow[:], 0.0)
        ones_col = wpool.tile([P, 1], F32, tag="ones_col")
        nc.vector.memset(ones_col[:], 1.0)
        # exp bias column for the 65-token tail tile: rows >= 65 get
        # exp(-huge) = 0 so its AV matmul can run at full K=128
        ebias = wpool.tile([P, 1], F32, tag="ebias")
        nc.vector.memset(ebias[:], NEG)
        nc.vector.memset(ebias[:TOK_TILES[4][1], :], 0.0)

        def load_x(b):
            outh, outl = [], []
            for i in range(CT):
                # [P, NPAD] with zero pad columns so M=128 tail matmuls
                # read zeros (re-zeroed per batch: fresh tile handles)
                t1 = xpool.tile([P, NPAD], F16, tag=f"xh{i}", name=f"xh{i}")
                nc.sync.dma_start(t1[:, :N],
                                  xh_e.ap()[b, i * P:(i + 1) * P, :])
                nc.vector.memset(t1[:, N:], 0.0)
                outh.append(t1)
            for i in range(CT):
                t2 = xpool.tile([P, N], BF16, tag=f"xl{i}", name=f"xl{i}")
                nc.sync.dma_start(t2[:],
                                  xl_e.ap()[b, i * P:(i + 1) * P, :])
                outl.append(t2)
            return outh, outl

        xh0, xl0 = load_x(0)
        load_weights()

        # v16: per token tile, head blocks of 128 cols:
        # [v_h (64) | ones | zeros (63)]
        v16 = [vpool.tile([P, H * P], F16, tag=f"v16_{mt}",
                          name=f"v16_{mt}") for mt in range(len(TOK_TILES))]
        for mt in range(len(TOK_TILES)):
            pad = v16[mt][:].rearrange("p (h c) -> p h c", h=H)[:, :, HD + 1:]
            nc.vector.memset(pad, 0.0)
            ones_ap = v16[mt][:].rearrange("p (h c) -> p h c",
                                           h=H)[:, :, HD:HD + 1]
            nc.vector.memset(ones_ap, 1.0)

        # kh16 zero pad columns; qpad per-head tiles fully zeroed once
        # (the per-batch casts only write head h's 64 rows, :N columns)
        kh16 = [qkpool.tile([P, NPAD], F16, tag=f"kh16_{mo}",
                            name=f"kh16_{mo}") for mo in range(CT)]
        for mo in range(CT):
            nc.vector.memset(kh16[mo][:, N:], 0.0)
        qpad = [qkpool.tile([P, NPAD], F16, tag=f"qpad{h}",
                            name=f"qpad{h}") for h in range(H)]
        for h in range(H):
            nc.vector.memset(qpad[h][:], 0.0)

        # o16 zero pad columns once (mults only write :N)
        o16 = [opool.tile([P, NPAD], F16, tag=f"o16_{i}", name=f"o16_{i}")
               for i in range(CT)]
        for i in range(CT):
            nc.vector.memset(o16[i][:, N:], 0.0)

        next_x = (xh0, xl0)
        for b in range(BL):
            xh_t, xl_t = next_x

            # ── K projection first: k32 (fp32) + kh16 (fp16) ─────────
            # (term-outer loops: one weight load serves both free chunks
            # of both x operands after Ldweights dedupe)
            k32 = []
            for mo in range(CT):
                ps = pspool.tile([P, C], F32, tag="bigps")
                for kc in range(CT):
                    first = kc == 0
                    last = kc == CT - 1
                    for (c0, cn) in _chunks(N):
                        nc.tensor.matmul(
                            ps[:, c0:c0 + cn],
                            lhsT=wkh_t[kc][:, mo * P:(mo + 1) * P],
                            rhs=xh_t[kc][:, c0:c0 + cn],
                            start=first, stop=False)
                    for (c0, cn) in _chunks(N):
                        nc.tensor.matmul(
                            ps[:, c0:c0 + cn],
                            lhsT=wkh_t[kc][:, mo * P:(mo + 1) * P],
                            rhs=xl_t[kc][:, c0:c0 + cn],
                            start=False, stop=False)
                    for (c0, cn) in _chunks(N):
                        nc.tensor.matmul(
                            ps[:, c0:c0 + cn],
                            lhsT=wkl_t[kc][:, mo * P:(mo + 1) * P],
                            rhs=xh_t[kc][:, c0:c0 + cn],
                            start=False, stop=last)
                sb32 = qkpool.tile([P, N], F32, tag=f"k32_{mo}",
                                   name=f"k32_{mo}")
                nc.scalar.copy(sb32[:], ps[:, :N])
                k32.append(sb32)
                nc.vector.tensor_copy(kh16[mo][:, :N], sb32[:])

            # ── Q projection: qkm (diag) + qpad casts, no q32 ────────
            sd_ps = psav.tile([P, NPAD], F32, tag="avps")
            for mo in range(CT):
                ps = pspool.tile([P, C], F32, tag="bigps")
                for kc in range(CT):
                    first = kc == 0
                    last = kc == CT - 1
                    for (c0, cn) in _chunks(N):
                        nc.tensor.matmul(
                            ps[:, c0:c0 + cn],
                            lhsT=wqh_t[kc][:, mo * P:(mo + 1) * P],
                            rhs=xh_t[kc][:, c0:c0 + cn],
                            start=first, stop=False)
                    for (c0, cn) in _chunks(N):
                        nc.tensor.matmul(
                            ps[:, c0:c0 + cn],
                            lhsT=wqh_t[kc][:, mo * P:(mo + 1) * P],
                            rhs=xl_t[kc][:, c0:c0 + cn],
                            start=False, stop=False)
                    for (c0, cn) in _chunks(N):
                        nc.tensor.matmul(
                            ps[:, c0:c0 + cn],
                            lhsT=wql_t[kc][:, mo * P:(mo + 1) * P],
                            rhs=xh_t[kc][:, c0:c0 + cn],
                            start=False, stop=last)
                # diag contribution: qkm = q*k elementwise (fp32 exact)
                qkm = qkpool.tile([P, NPAD], F32, tag="bigscratch", bufs=2)
                nc.vector.tensor_mul(qkm[:, :N], ps[:, :N], k32[mo][:])
                for (c0, cn) in _chunks(N):
                    nc.tensor.matmul(
                        sd_ps[:H, c0:c0 + cn],
                        lhsT=hsel_t[mo][:],
                        rhs=qkm[:, c0:c0 + cn],
                        start=(mo == 0), stop=(mo == CT - 1))
                # per-head zero-padded q tiles (other head's rows stay 0)
                nc.vector.tensor_copy(qpad[2 * mo][:HD, :N], ps[:HD, :N])
                nc.vector.tensor_copy(qpad[2 * mo + 1][HD:, :N],
                                      ps[HD:, :N])
            sd_sb = spool.tile([H, N], F32, tag="sd_sb")
            nc.scalar.copy(sd_sb[:], sd_ps[:H, :N])

            # ── V projection (fp16) → v16[mt] 128-wide head blocks ───
            for mt, (t0, tn) in enumerate(TOK_TILES):
                ps = pspool.tile([P, C], F32, tag="bigps")
                for kc in range(CT):
                    for (c0, cn) in _chunks(C):
                        nc.tensor.matmul(
                            ps[:, c0:c0 + cn],
                            lhsT=xh_t[kc][:, t0:t0 + P],
                            rhs=wv_t[kc][:, c0:c0 + cn],
                            start=(kc == 0), stop=(kc == CT - 1))
                dst = v16[mt][:, :].rearrange("p (h c) -> p h c",
                                              h=H)[:, :, :HD]
                src = ps[:, :C].rearrange("p (h c) -> p h c", h=H)
                nc.scalar.copy(dst, src)
            # prefetch next batch's x as soon as this batch's is consumed
            if b + 1 < BL:
                next_x = load_x(b + 1)

            # ── per-head attention ───────────────────────────────────
            rowsum_all = spool.tile([H, N], F32, tag="rowsum_all")

            def head_scores(h):
                # scores: K=128 (zero-padded q), M=128 (zero-padded keys)
                mo = h // 2
                p16 = []
                for mt, (t0, tn) in enumerate(TOK_TILES):
                    ps = pspool.tile([P, C], F32, tag="bigps")
                    for (c0, cn) in _chunks(N):
                        nc.tensor.matmul(
                            ps[:, c0:c0 + cn],
                            lhsT=kh16[mo][:, t0:t0 + P],
                            rhs=qpad[h][:, c0:c0 + cn],
                            start=True, stop=True)
                    pt = ppool.tile([P, N], F16, tag=f"p16_{mt}",
                                    name=f"p16_{mt}", bufs=2)
                    if tn == P:
                        nc.scalar.activation(pt[:], ps[:, :N],
                                             ACTF.Exp, scale=SCALE)
                    else:
                        # tail tile: bias forces rows >= tn to exp(..)=0,
                        # keeping the full 128 rows valid for K=128 AV
                        nc.scalar.activation(pt[:], ps[:, :N],
                                             ACTF.Exp, scale=SCALE,
                                             bias=ebias[:])
                    p16.append(pt)
                return p16

            def head_av(h, p16):
                # attn @ [V|1|0..] → av rows 0:64, rowsum row 64
                mo, r0 = h // 2, (h % 2) * HD
                av_ps = psav.tile([P, NPAD], F32, tag="avps")
                for mt, (t0, tn) in enumerate(TOK_TILES):
                    for (c0, cn) in _chunks(N):
                        nc.tensor.matmul(
                            av_ps[:, c0:c0 + cn],
                            lhsT=v16[mt][:, h * P:(h + 1) * P],
                            rhs=p16[mt][:, c0:c0 + cn],
                            start=(mt == 0),
                            stop=(mt == len(TOK_TILES) - 1))
                # rowsum row 64: aligned DVE copy out of PSUM, DMA hop to
                # partition 0 + into rowsum_all (custom-DVE reciprocal
                # mishandles PSUM/base-64 operands, so hop first). Output
                # normalize uses the fast reciprocal (1e-3 budget); the
                # ranking redoes an accurate one on rowsum_all.
                rs64 = spool.tile([P, N], F32, tag=f"rs64_{h % 2}",
                                  name=f"rs64_{h % 2}")
                nc.vector.tensor_single_scalar(
                    rs64[HD:HD + 1, :], av_ps[HD:HD + 1, :N], 1.0, OP.mult)
                nc.scalar.dma_start(rowsum_all[h:h + 1, :],
                                    rs64[HD:HD + 1, :])
                rs0 = spool.tile([1, N], F32, tag="rs0", name="rs0")
                nc.scalar.dma_start(rs0[:], rs64[HD:HD + 1, :])
                rec_h = spool.tile([1, N], F32, tag=f"rec{h % 2}",
                                   name=f"rec{h % 2}")
                nc.vector.reciprocal_approx_fast(rec_h[:], rs0[:])
                bc = spool.tile([P, N], F32, tag=f"bc{h % 2}",
                                name=f"bc{h % 2}")
                nc.gpsimd.partition_broadcast(
                    bc[:HD, :], rec_h[:], channels=HD)
                nc.vector.tensor_tensor(
                    o16[mo][r0:r0 + HD, :N], av_ps[:HD, :N],
                    bc[:HD, :], OP.mult)

            for h in range(H):
                head_av(h, head_scores(h))

            def emit_yproj():
                # ── output projection + residual (PE keeps running) ──
                for mt, (t0, tn) in enumerate(TOK_TILES):
                    y_ps = pspool.tile([P, C], F32, tag="bigps")
                    for kc in range(CT):
                        for (c0, cn) in _chunks(C):
                            nc.tensor.matmul(
                                y_ps[:, c0:c0 + cn],
                                lhsT=o16[kc][:, t0:t0 + P],
                                rhs=wp_t[kc][:, c0:c0 + cn],
                                start=(kc == 0), stop=(kc == CT - 1))
                    xr_t = ypool.tile([P, C], F32, tag="xr_t", bufs=2)
                    nc.sync.dma_start(xr_t[:tn, :], xr_e.ap()[b, t0:t0 + tn, :])
                    y1 = ypool.tile([P, C], F32, tag=f"y1_{mt}",
                                    name=f"y1_{mt}", bufs=1)
                    # ACT copy frees the PSUM buffer without waiting on the
                    # DVE queue; the residual add then runs in place
                    nc.scalar.copy(y1[:tn, :], y_ps[:tn, :])
                    nc.vector.tensor_add(y1[:tn, :], y1[:tn, :],
                                         xr_t[:tn, :])
                    y1s.append(y1)

            def emit_rank():
                # ── ranking chain (DVE/GpSimd/DMA only — no PE stalls)
                pd = spool.tile([H, N], F32, tag="pd")
                nc.scalar.activation(pd[:], sd_sb[:], ACTF.Exp, scale=SCALE)
                rrec = spool.tile([H, N], F32, tag="rrec")
                rscr = spool.tile([H, N], F32, tag="a_red", name="rscr2")
                nc.vector.reciprocal_approx_accurate(rrec[:], rowsum_all[:],
                                                     rscr[:])
                nc.vector.tensor_mul(pd[:], pd[:], rrec[:])
                a_red = spool.tile([H, N], F32, tag="a_red")
                nc.gpsimd.partition_all_reduce(
                    a_red[:], pd[:], channels=H,
                    reduce_op=bass_isa.ReduceOp.add)
                a_row = spool.tile([1, NPAD], F32, tag="a_row")
                nc.vector.tensor_copy(a_row[:, :N], a_red[0:1, :])
                nc.vector.memset(a_row[:, N:], NEG)
                nc.vector.memset(a_row[:, 0:1], 1.0e30)   # CLS always kept

                abc = spool.tile([P, NPAD], F32, tag="abc")
                nc.gpsimd.partition_broadcast(abc[:], a_row[:])
                a_dram = dpool.tile([1, NPAD], F32, tag="a_dram")
                nc.scalar.dma_start(a_dram[:], a_row[:])
                acp = spool.tile([P, 5], F32, tag="acp")
                nc.scalar.dma_start(
                    acp[:], a_dram[:, :].rearrange("a (t p) -> (a p) t", p=P))
                rcnt = spool.tile([P, 5], F32, tag="rcnt")
                scratch = qkpool.tile([P, NPAD], F32, tag="bigscratch",
                                      bufs=2)
                keep = spool.tile([P, 5], F32, tag="keep")
                for t in range(5):
                    nc.vector.tensor_scalar(
                        scratch[:], abc[:], acp[:, t:t + 1], None, OP.is_gt,
                        op1=OP.add, accum_out=rcnt[:, t:t + 1])
                    nc.vector.tensor_single_scalar(
                        keep[:, t:t + 1], rcnt[:, t:t + 1], float(num_kept),
                        OP.is_lt)
                # positions in column form: PE triangular-ones matmul gives
                # the within-tile prefix over partitions; a ones-column
                # matmul gives per-tile totals for the cross-tile offsets.
                # token id of (p, t) is t*128 + p, so pos(p,t) =
                # colprefix_incl(p,t) + sum_{t'<t} total(t').
                pos_ps = psav.tile([P, NPAD], F32, tag="avps")
                nc.tensor.matmul(pos_ps[:, 0:5], lhsT=tri_box[0][:],
                                 rhs=keep[:], start=True, stop=True)
                nc.tensor.matmul(pos_ps[:1, 8:13], lhsT=ones_col[:],
                                 rhs=keep[:], start=True, stop=True)
                posc = spool.tile([P, 5], F32, tag="posc")
                nc.scalar.copy(posc[:], pos_ps[:, 0:5])
                srow = spool.tile([1, 5], F32, tag="srow")
                nc.scalar.copy(srow[:], pos_ps[:1, 8:13])
                sinc = spool.tile([1, 5], F32, tag="sinc")
                nc.vector.tensor_tensor_scan(
                    sinc[:], srow[:], zrow[:, :5], 0.0, OP.add, OP.add)
                offs = spool.tile([1, 5], F32, tag="offs")
                nc.vector.tensor_tensor(offs[:], sinc[:], srow[:],
                                        OP.subtract)
                offsb = spool.tile([P, 5], F32, tag="offsb")
                nc.gpsimd.partition_broadcast(offsb[:], offs[:])
                # index = colprefix + offs + (b*num_kept - 1), dropped->BIG
                pos_col = spool.tile([P, 5], F32, tag="pos_col")
                nc.vector.tensor_tensor(pos_col[:], posc[:], offsb[:],
                                        OP.add)
                nc.vector.tensor_single_scalar(
                    pos_col[:], pos_col[:], float(b * num_kept - 1), OP.add)
                nc.vector.tensor_scalar(
                    keep[:], keep[:], -BIG, BIG, OP.mult, op1=OP.add)
                nc.vector.tensor_tensor(pos_col[:], pos_col[:], keep[:],
                                        OP.add)
                icpu = spool.tile([P, 5], U32, tag="icpu")
                nc.vector.tensor_copy(icpu[:], pos_col[:])
                icpu_box[0] = icpu

            def emit_scatter():
                for mt, (t0, tn) in enumerate(TOK_TILES):
                    nc.gpsimd.indirect_dma_start(
                        out=out_flat,
                        out_offset=bass.IndirectOffsetOnAxis(
                            ap=icpu_box[0][:tn, mt:mt + 1], axis=0),
                        in_=y1s[mt][:tn, :],
                        in_offset=None,
                        bounds_check=BL * num_kept - 1,
                        oob_is_err=False)

            y1s = []
            icpu_box = [None]
            if b == BL - 1:
                # last batch: rank chain first so the final scatters
                # overlap the output projection instead of trailing it
                emit_rank()
                emit_yproj()
            else:
                emit_yproj()
                emit_rank()
            emit_scatter()

    _dedupe_ldweights(nc)
    nc.compile()
    return nc


def prep_inputs(x, qkv_w, proj_w, proj_b):
    """Host-side sharding + layout prep. Returns per-core in_maps."""
    import ml_dtypes
    bf16 = ml_dtypes.bfloat16

    x = np.ascontiguousarray(x, dtype=np.float32)
    qkv_w = np.asarray(qkv_w, dtype=np.float32)
    proj_w = np.asarray(proj_w, dtype=np.float32)
    proj_b = np.asarray(proj_b, dtype=np.float32)

    wq = np.ascontiguousarray(qkv_w[0:C].T)           # (c_in, c_out)
    wk = np.ascontiguousarray(qkv_w[C:2 * C].T)
    wqh = wq.astype(np.float16)
    wql = (wq - wqh.astype(np.float32)).astype(bf16)
    wkh = wk.astype(np.float16)
    wkl = (wk - wkh.astype(np.float32)).astype(bf16)
    wv16 = np.ascontiguousarray(qkv_w[2 * C:3 * C].T).astype(np.float16)
    wp16 = np.ascontiguousarray(proj_w.T).astype(np.float16)
    hsel = np.zeros((C, H), dtype=np.float32)
    for h in range(H):
        hsel[h * HD:(h + 1) * HD, h] = 1.0
    tri = np.triu(np.ones((P, P), dtype=np.float32))  # tri[i,j]=1 iff i<=j

    in_maps = []
    for core in range(NCORES):
        xl_ = x[core * BL:(core + 1) * BL]            # (BL, N, C)
        xt = np.ascontiguousarray(xl_.transpose(0, 2, 1))  # (BL, C, N)
        xth = xt.astype(np.float16)
        xtl = (xt - xth.astype(np.float32)).astype(bf16)
        in_maps.append({
            "xh": xth,
            "xl": xtl,
            "xr": np.ascontiguousarray(xl_ + proj_b[None, None, :]),
            "wqh": wqh, "wql": wql, "wkh": wkh, "wkl": wkl,
            "wv16": wv16, "wp16": wp16, "hsel": hsel, "tri": tri,
        })
    return in_maps


_BUILD_CACHE = {}


def run(x, qkv_w, proj_w, proj_b, reduction_num, trace=False, **trace_kw):
    num_kept = N - int(reduction_num)
    if num_kept not in _BUILD_CACHE:
        _BUILD_CACHE[num_kept] = build(num_kept)
    nc = _BUILD_CACHE[num_kept]
    in_maps = prep_inputs(x, qkv_w, proj_w, proj_b)
    res = run_bass_kernel_spmd(nc, in_maps, core_ids=list(range(NCORES)),
                               trace=trace, **trace_kw)
    out = np.concatenate([res.results[c]["out"] for c in range(NCORES)],
                         axis=0)
    return out.astype(np.float32), res


def kernel(x, qkv_w, proj_w, proj_b, reduction_num):
    out, _ = run(x, qkv_w, proj_w, proj_b, reduction_num, trace=False)
    return out
